# revision 25
# baseline (speedup 1.0000x reference)
"""CWCFace head (nn_CWCFace_11201274708637) — Trainium2 Bass kernel.

Math (reference):
    kn = kernel / ||kernel||_col
    cos = clip(emb @ kn, -1+eps, 1-eps)              # [B, C]
    ms  = margin_scaler(norms, label)                # [B, 1] per-sample stats
    th  = arccos(cos); th_m = clip(th + onehot*(-M*ms), eps, pi-eps)
    out = (cos(th_m) - onehot*(M + M*ms)) * S

Split of work:
  - Device (the O(B*EMB*C) part): out = clip(embT.T @ K', +-S*(1-eps))
    where K' = S * kernel / ||kernel||_col is folded into the bf16 kernel
    upload.  Output is stored bf16 (quantization ~2^-9 rel, well under the
    tolerance) which halves store traffic.
  - Host (the O(B) part): per-class segment stats of the safe norms and
    the one-hot margin fix-up touch exactly one column per row; the B=512
    corrected entries are computed exactly in float64 from the raw inputs
    and overwrite out[i, label_i] after the gather.

Sharding: classes column-split over 8 cores, CS = 8848 each (8*8848 =
70784 >= 70722).  Per core the device kernel is a pure stream:
  for each 1024-wide class block: DMA kernel block -> 4x4 [128,128]x[128,W]
  bf16 matmuls per 512-wide slice -> one DVE tensor_scalar (max,min) clip
  from PSUM straight to a bf16 staging tile -> 2KB-per-row DMA store.
PE busy ~69us is the roofline for bf16 at this shape; DMA in+out is
~18.6MB (~53us), so PE is the (slightly) binding engine of this ridge.
"""

import sys

for _p in (
    "/root/.axon_site",
    "/root/.axon_site/_ro/trn_rl_repo",
    "/root/.axon_site/_ro/pypackages",
    "/opt/trn_rl_repo",
):
    if _p not in sys.path:
        sys.path.append(_p)

import math

import numpy as np

import concourse.bass as bass
import concourse.mybir as mybir
import concourse.tile as tile
from concourse import bacc
from concourse.bass_utils import run_bass_kernel_spmd

B = 512
EMB = 512
C = 70722
NCORES = 8
CS = 8848  # per-core classes (padded);  8 * 8848 = 70784 >= 70722
S = 64.0
MARG = 0.4
H = 0.333
EPS = 1e-3

F32 = mybir.dt.float32
BF16 = mybir.dt.bfloat16
AL = mybir.AluOpType

KT = EMB // 128          # 4 K-tiles
BT = B // 128            # 4 B-tiles
CLIP = S * (1.0 - EPS)


PAIR_WIDTHS = [256, 768] + [1024] * 6 + [512, 656, 368, 144]
assert sum(PAIR_WIDTHS) == CS
N_WARM = 10  # PE warm-up matmuls bridging the DVFS ramp until data lands


def _pairs():
    """Class-column blocks per core: (c0, [slice widths]).  First block is
    small so PE starts early; mid blocks are 1024 (2KB store rows); the
    tail tapers so the final store drains fast."""
    out = []
    c0 = 0
    for wb in PAIR_WIDTHS:
        ws = []
        off = 0
        while off < wb:
            w = min(512, wb - off)
            ws.append(w)
            off += w
        out.append((c0, ws))
        c0 += wb
    return out


def _emit(nc, tc, embT_h, kern_h, out_h):
    # host pre-arranges embT/kern so every load is contiguous per
    # partition row (4-8KB DMA descriptors instead of 256B-2KB strided)
    embR = embT_h[:, :].rearrange("p (k b) -> p k b", k=KT)  # [128, KT, B]
    # row r = b*128 + p  ->  [p, b, c] view for one store per pair
    outR = out_h[:, :].rearrange("(b p) c -> p b c", p=128)

    cst_cm = tc.tile_pool(name="cst", bufs=1)
    cst = cst_cm.__enter__()
    embT_sb = cst.tile([128, KT, B], BF16, tag="embT")  # [p, k, b]
    warm_sb = cst.tile([128, 512], BF16, tag="warm")

    pairs = _pairs()
    with (
        tc.tile_pool(name="kp", bufs=5) as kp,
        tc.tile_pool(name="st", bufs=4) as st,
        tc.tile_pool(name="ps", bufs=8, space="PSUM") as ps,
    ):
        # PE warm-up: dep-free matmuls on scratch data keep the PE busy
        # (and its clock ramped) while the first real tiles stream in.
        nc.gpsimd.memset(warm_sb[:], 1.0)
        ps_warm = ps.tile([128, 512], F32, space="PSUM", tag="po")
        for i in range(N_WARM):
            nc.tensor.matmul(
                ps_warm[:],
                warm_sb[:, 0:128],
                warm_sb[:],
                start=(i == 0),
                stop=(i == N_WARM - 1),
            )

        # Loads on the sync HWDGE ring (embT first — it gates the first
        # matmul); stores on the scalar ring so store doorbells and their
        # waits never delay load issuance.
        def load_pair(c0, ws):
            Wb = sum(ws)
            ksb = kp.tile([128, KT, Wb], BF16, tag="ks")
            # pair-major host layout: pair block at column KT*c0, contiguous
            nc.sync.dma_start(
                out=ksb[:],
                in_=kern_h[:, KT * c0 : KT * (c0 + Wb)].rearrange(
                    "p (k w) -> p k w", k=KT
                ),
            )
            return ksb

        nc.sync.dma_start(out=embT_sb[:], in_=embR)
        ksbs = [load_pair(*pairs[0]), load_pair(*pairs[1])]

        for i, (c0, widths) in enumerate(pairs):
            if i + 2 < len(pairs):
                ksbs.append(load_pair(*pairs[i + 2]))
            ksb = ksbs[i]
            Wb = sum(widths)
            stg = st.tile([128, BT, Wb], BF16, tag="stg")
            for b in range(BT):
                off = 0
                for w in widths:
                    psb = ps.tile([128, w], F32, space="PSUM", tag="po")
                    for k in range(KT):
                        nc.tensor.matmul(
                            psb[:],
                            embT_sb[:, k, b * 128 : (b + 1) * 128],
                            ksb[:, k, off : off + w],
                            start=(k == 0),
                            stop=(k == KT - 1),
                        )
                    nc.vector.tensor_scalar(
                        stg[:, b, off : off + w],
                        psb[:],
                        -CLIP,
                        CLIP,
                        op0=AL.max,
                        op1=AL.min,
                    )
                    off += w
            nc.scalar.dma_start(
                out=outR[:, :, c0 : c0 + Wb],
                in_=stg[:],
            )

    cst_cm.__exit__(None, None, None)


def _build():
    nc = bacc.Bacc(
        "TRN2", target_bir_lowering=False, debug=False, num_devices=NCORES
    )
    embT_h = nc.dram_tensor("embT", [128, KT * B], BF16, kind="ExternalInput")
    kern_h = nc.dram_tensor("kern", [128, KT * CS], BF16, kind="ExternalInput")
    out_h = nc.dram_tensor("out", [B, CS], BF16, kind="ExternalOutput")
    with tile.TileContext(nc) as tc:
        _emit(nc, tc, embT_h, kern_h, out_h)
    nc.compile()
    return nc


_NC = None


def _get_nc():
    global _NC
    if _NC is None:
        _NC = _build()
    return _NC


def _prep_inputs(embbedings, norms, label, kernel):
    import ml_dtypes

    bf16 = ml_dtypes.bfloat16
    # embT[p, k*B + b] = emb[b, k*128 + p]: contiguous 4KB rows on device
    embT = (
        np.asarray(embbedings, dtype=np.float32)
        .T.reshape(KT, 128, B)
        .transpose(1, 0, 2)
        .reshape(128, KT * B)
        .astype(bf16)
    )
    embT = np.ascontiguousarray(embT)
    kern = np.asarray(kernel, dtype=np.float32)
    cn = np.sqrt(np.einsum("ij,ij->j", kern, kern, dtype=np.float64))
    kscaled = (kern * (S / cn)[None, :].astype(np.float32)).astype(bf16)
    kern_pad = np.zeros((EMB, CS * NCORES), dtype=bf16)
    kern_pad[:, :C] = kscaled
    # [p, k, c] with pair-major blocks: each pair load is one contiguous
    # [128, KT*Wb] region (8KB descriptors for 1024-wide pairs)
    kern_pkc = kern_pad.reshape(KT, 128, CS * NCORES).transpose(1, 0, 2)
    offs = np.cumsum([0] + PAIR_WIDTHS[:-1]).tolist()
    in_maps = []
    for c in range(NCORES):
        blocks = [
            np.ascontiguousarray(
                kern_pkc[:, :, c * CS + c0 : c * CS + c0 + w]
            ).reshape(128, KT * w)
            for c0, w in zip(offs, PAIR_WIDTHS)
        ]
        in_maps.append(
            {
                "embT": embT,
                "kern": np.ascontiguousarray(np.concatenate(blocks, axis=1)),
            }
        )
    return in_maps


def _run(in_maps, **kwargs):
    nc = _get_nc()
    return run_bass_kernel_spmd(nc, in_maps, core_ids=list(range(NCORES)), **kwargs)


def _fixup(out, embbedings, norms, label, kernel):
    """Exact (f64) one-hot margin correction: out[i, label_i]."""
    emb = np.asarray(embbedings, dtype=np.float64)
    kern = np.asarray(kernel, dtype=np.float64)
    lab = np.asarray(label).astype(np.int64).reshape(B)
    v = np.clip(np.asarray(norms, dtype=np.float64).reshape(B), 0.001, 100.0)

    cnt = np.bincount(lab, minlength=C).astype(np.float64)
    ssum = np.bincount(lab, weights=v, minlength=C)
    ssq = np.bincount(lab, weights=v * v, minlength=C)
    n = cnt[lab]
    mean = ssum[lab] / n
    var = (ssq[lab] - n * mean * mean) / np.maximum(n - 1.0, 1.0)
    std = np.sqrt(np.maximum(var, 0.0))
    res = np.where(n > 2.0, (v - mean) / (std + EPS), (v - mean) / 20.0)
    ms = np.clip(res * H, -1.0, 1.0)

    kcol = kern[:, lab]  # [EMB, B]
    t = np.einsum("bi,ib->b", emb, kcol) / np.sqrt(
        np.einsum("ib,ib->b", kcol, kcol)
    )
    t = np.clip(t, -1.0 + EPS, 1.0 - EPS)
    theta = np.arccos(t)
    theta_m = np.clip(theta - MARG * ms, EPS, math.pi - EPS)
    val = (np.cos(theta_m) - (MARG + MARG * ms)) * S
    out[np.arange(B), lab] = val.astype(np.float32)


def kernel(embbedings, norms, label, kernel):
    in_maps = _prep_inputs(embbedings, norms, label, kernel)
    res = _run(in_maps)
    parts = [
        np.asarray(res.results[c]["out"]).reshape(B, CS) for c in range(NCORES)
    ]
    out = np.concatenate(parts, axis=1)[:, :C].astype(np.float32)
    _fixup(out, embbedings, norms, label, kernel)
    return out
